# revision 1
# baseline (speedup 1.0000x reference)
"""Self-contained Trainium2 Bass kernel for nn_ComplementarityScoreHead.

out = (h_norm @ h_norm.T) * edge_mask, h = MLP(x), h_norm = h / ||h||_2(rows)

Strategy (8 NeuronCores, SPMD):
  - Each core m receives x rolled by -1024*m rows, so its 1024-row output
    slab always sits at local columns [0:1024) of the on-chip transposed
    feature matrix; one identical program runs on all cores.
  - Per core: fused per-512-column tile: PE-transpose of x, MLP layer1
    (relu) and layer2 (fp32r matmuls), sum-of-squares over the feature
    (partition) axis via an all-ones [128,128] matmul (result lands
    row-broadcast in PSUM), rsqrt, and normalization into hnT (fp32r).
  - Correlation slab computed 128 rows at a time: [128,512] fp32r matmuls
    accumulate K=256 into PSUM; the edge mask chunk is built on GPSIMD with
    local_scatter (per-partition column indices, bf16 ones); the PSUM
    eviction fuses the mask multiply; dense 4 MiB chunks stream to DRAM.
  - Host glue: dedup edges, bucket them by (core, chunk, window, row) into
    padded int16 index tables, roll x per core, un-roll output columns.
"""
import sys
import numpy as np

sys.path.insert(0, '/opt/trn_rl_repo')

import concourse.bass as bass  # noqa: E402
import concourse.mybir as mybir  # noqa: E402
from concourse import bacc  # noqa: E402
from concourse.tile import TileContext  # noqa: E402
from concourse.masks import make_identity  # noqa: E402
from concourse.bass_utils import run_bass_kernel_spmd  # noqa: E402

N = 8192
F = 128
H = 256
NCORES = 8
SLAB = N // NCORES
CHUNKS = SLAB // 128
NT = N // 512


NG = 4
GW = N // NG


def _col_windows():
    # Per 2048-wide column group: one 2046-wide + one 2-wide scatter window
    # (only hardware-verified local_scatter num_elems values).
    wins = []
    for g in range(NG):
        wins.append((g * GW, GW - 2))
        wins.append((g * GW + GW - 2, 2))
    return wins


WINS = _col_windows()
NW = len(WINS)


def _build_nc(K_pad, chunk_splits=1, chunk_bufs=2, mask_bufs=2):
    f32 = mybir.dt.float32
    f32r = mybir.dt.float32r
    bf16 = mybir.dt.bfloat16

    nc = bacc.Bacc()
    x = nc.declare_dram_parameter("x", [N, F], f32, isOutput=False)
    W1 = nc.declare_dram_parameter("W1", [F, H], f32, isOutput=False)
    b1 = nc.declare_dram_parameter("b1", [128, 2], f32, isOutput=False)
    W2 = nc.declare_dram_parameter("W2", [128, 2, H], f32, isOutput=False)
    b2 = nc.declare_dram_parameter("b2", [128, 2], f32, isOutput=False)
    idx = nc.declare_dram_parameter("idx", [128, CHUNKS * NW * K_pad],
                                    mybir.dt.int16, isOutput=False)
    out = nc.declare_dram_parameter("out", [SLAB, N], f32, isOutput=True)

    with TileContext(nc) as tc:
        with (
            tc.tile_pool(name="singles", bufs=1) as singles,
            tc.tile_pool(name="hn", bufs=2) as hn_pool,
            tc.tile_pool(name="psA", bufs=4, space="PSUM") as psA,
            tc.tile_pool(name="psW", bufs=2, space="PSUM") as psW,
        ):
            ident = singles.tile([128, 128], f32)
            make_identity(nc, ident[:])
            w1f = singles.tile([128, H], f32)
            nc.sync.dma_start(out=w1f[:], in_=W1[:])
            w1r = singles.tile([128, H], f32r)
            nc.vector.tensor_copy(w1r[:], w1f[:])
            w2f = singles.tile([128, 2, H], f32)
            nc.sync.dma_start(out=w2f[:], in_=W2[:])
            w2r = singles.tile([128, 2, H], f32r)
            nc.vector.tensor_copy(w2r[:], w2f[:])
            b1s = singles.tile([128, 2], f32)
            nc.sync.dma_start(out=b1s[:], in_=b1[:])
            b2s = singles.tile([128, 2], f32)
            nc.sync.dma_start(out=b2s[:], in_=b2[:])
            idx_sb = singles.tile([128, CHUNKS, NW, K_pad], mybir.dt.int16)
            nc.sync.dma_start(
                out=idx_sb[:],
                in_=idx.rearrange("p (c w k) -> p c w k", c=CHUNKS, w=NW),
            )
            ones_bf = singles.tile([128, K_pad], bf16)
            nc.vector.memset(ones_bf[:], 1.0)
            ones_sq_f = singles.tile([128, 128], f32)
            nc.vector.memset(ones_sq_f[:], 1.0)
            ones_sq = singles.tile([128, 128], f32r)
            nc.vector.tensor_copy(ones_sq[:], ones_sq_f[:])

            hnT = [hn_pool.tile([128, N], f32r, tag="hn", name=f"hnT{s}")
                   for s in range(2)]

            with (
                tc.tile_pool(name="xa", bufs=6) as xa_pool,
                tc.tile_pool(name="mid", bufs=3) as mid,
            ):
                x_t = x.rearrange("(t p) f -> t p f", p=128)
                for nt in range(NT):
                    sl = slice(nt * 512, (nt + 1) * 512)
                    xts = mid.tile([128, 512], f32r, tag="xts")
                    xa = xa_pool.tile([128, 4, F], f32, tag="xa")
                    dmae = nc.sync if nt % 2 == 0 else nc.scalar
                    dmae.dma_start(out=xa[:], in_=x_t[nt * 4:(nt + 1) * 4].rearrange("t p f -> p t f"))
                    ptx = psA.tile([128, 512], f32, tag="ps")
                    for u in range(4):
                        nc.tensor.transpose(ptx[:, u * 128:(u + 1) * 128], xa[:, u, :], ident[:])
                    nc.vector.tensor_copy(xts[:], ptx[:])
                    r1s = mid.tile([128, 2, 512], f32r, tag="r1s")
                    for s in range(2):
                        ps = psA.tile([128, 512], f32, tag="ps")
                        nc.tensor.matmul(
                            ps[:], w1r[:, s * 128:(s + 1) * 128], xts[:],
                            start=True, stop=True)
                        nc.scalar.activation(
                            r1s[:, s, :], ps[:],
                            mybir.ActivationFunctionType.Relu,
                            bias=b1s[:, s:s + 1])
                    hts = mid.tile([128, 2, 512], f32, tag="hts")
                    for s2 in range(2):
                        ps = psA.tile([128, 512], f32, tag="ps")
                        for k in range(2):
                            nc.tensor.matmul(
                                ps[:], w2r[:, k, s2 * 128:(s2 + 1) * 128],
                                r1s[:, k, :], start=(k == 0), stop=(k == 1))
                        nc.scalar.activation(
                            hts[:, s2, :], ps[:],
                            mybir.ActivationFunctionType.Identity,
                            bias=b2s[:, s2:s2 + 1])
                    pss = psA.tile([128, 512], f32, tag="ps")
                    for s in range(2):
                        sqs = mid.tile([128, 512], f32r, tag="sqs")
                        nc.gpsimd.tensor_mul(sqs[:], hts[:, s, :], hts[:, s, :])
                        nc.tensor.matmul(
                            pss[:], ones_sq[:], sqs[:],
                            start=(s == 0), stop=(s == 1))
                    rsq = mid.tile([128, 512], f32, tag="rsq")
                    nc.scalar.activation(rsq[:], pss[:],
                                         mybir.ActivationFunctionType.Sqrt)
                    nc.vector.reciprocal(rsq[:], rsq[:])
                    for s in range(2):
                        eng = nc.gpsimd if s == 0 else nc.vector
                        eng.tensor_mul(hnT[s][:, sl], hts[:, s, :], rsq[:])

            with tc.tile_pool(name="chunkh", bufs=3) as chunk_pool, \
                 tc.tile_pool(name="maskh", bufs=3) as mask_pool:
                for g in range(NG):
                    for mt in range(CHUNKS):
                        maskh = mask_pool.tile([128, GW], bf16, tag="maskh")
                        for wi in range(2):
                            w = g * 2 + wi
                            woff, wlen = WINS[w]
                            nc.gpsimd.local_scatter(
                                maskh[:, woff - g * GW: woff - g * GW + wlen],
                                ones_bf[:],
                                idx_sb[:, mt, w, :], channels=128,
                                num_elems=wlen, num_idxs=K_pad)
                        chunkh = chunk_pool.tile([128, GW], f32, tag="chunkh")
                        for ntw in range(GW // 1024):
                            ps = psW.tile([128, 1024], f32, tag="psw")
                            for sub in range(2):
                                nt = (g * GW + ntw * 1024 + sub * 512) // 512
                                for k in range(2):
                                    nc.tensor.matmul(
                                        ps[:, sub * 512:(sub + 1) * 512],
                                        hnT[k][:, mt * 128:(mt + 1) * 128],
                                        hnT[k][:, nt * 512:(nt + 1) * 512],
                                        start=(k == 0), stop=(k == 1))
                            nc.vector.tensor_mul(
                                chunkh[:, ntw * 1024:(ntw + 1) * 1024], ps[:],
                                maskh[:, ntw * 1024:(ntw + 1) * 1024])
                        qsel = (g * CHUNKS + mt) % 2
                        dmao = nc.sync if qsel == 0 else nc.scalar
                        dmao.dma_start(
                            out=out[mt * 128:(mt + 1) * 128, g * GW:(g + 1) * GW],
                            in_=chunkh[:])
    nc.compile()
    return nc


def _prep_edges(edge_index, K_pad=None):
    r = np.asarray(edge_index[0], dtype=np.int64)
    c = np.asarray(edge_index[1], dtype=np.int64)
    flat = np.unique(r * N + c)
    r = flat // N
    c = flat % N
    core = r // SLAB
    base = core * SLAB
    p_full = r - base
    chunk = p_full // 128
    prow = p_full % 128
    j = (c - base) % N
    wbound = np.array([w[0] for w in WINS] + [N])
    wid = np.searchsorted(wbound, j, side="right") - 1
    woff = j - wbound[wid]
    key = ((core * CHUNKS + chunk) * 128 + prow) * NW + wid
    order = np.argsort(key, kind="stable")
    key_s = key[order]
    woff_s = woff[order]
    ncells = NCORES * CHUNKS * 128 * NW
    counts = np.bincount(key_s, minlength=ncells)
    kmax = int(counts.max())
    if K_pad is None:
        K_pad = max(2, (kmax + 1) // 2 * 2)
    assert kmax <= K_pad, (kmax, K_pad)
    idx_all = np.full((ncells, K_pad), -1, dtype=np.int16)
    starts = np.zeros(ncells + 1, np.int64)
    np.cumsum(counts, out=starts[1:])
    cell_ids = np.repeat(np.arange(ncells), counts)
    pos = np.arange(len(key_s)) - starts[cell_ids]
    idx_all[cell_ids, pos] = woff_s.astype(np.int16)
    idx_all = idx_all.reshape(NCORES, CHUNKS, 128, NW, K_pad)
    idx_all = np.ascontiguousarray(idx_all.transpose(0, 2, 1, 3, 4))
    return idx_all.reshape(NCORES, 128, CHUNKS * NW * K_pad), K_pad


_NC_CACHE = {}


def kernel(x, edge_index, W1, b1, W2, b2):
    x = np.ascontiguousarray(np.asarray(x, dtype=np.float32))
    W1 = np.ascontiguousarray(np.asarray(W1, dtype=np.float32))
    W2h = np.ascontiguousarray(
        np.asarray(W2, dtype=np.float32).reshape(2, 128, H).transpose(1, 0, 2))
    b1h = np.ascontiguousarray(np.asarray(b1, dtype=np.float32).reshape(2, 128).T)
    b2h = np.ascontiguousarray(np.asarray(b2, dtype=np.float32).reshape(2, 128).T)
    idx_all, K_pad = _prep_edges(edge_index)

    if K_pad not in _NC_CACHE:
        _NC_CACHE[K_pad] = _build_nc(K_pad)
    nc = _NC_CACHE[K_pad]

    in_maps = []
    for m in range(NCORES):
        xm = np.ascontiguousarray(np.roll(x, -SLAB * m, axis=0))
        in_maps.append({"x": xm, "W1": W1, "b1": b1h, "W2": W2h, "b2": b2h,
                        "idx": np.ascontiguousarray(idx_all[m])})

    res = run_bass_kernel_spmd(nc, in_maps, list(range(NCORES)))

    out = np.empty((N, N), dtype=np.float32)
    for m in range(NCORES):
        out[m * SLAB:(m + 1) * SLAB] = np.roll(res.results[m]["out"],
                                               SLAB * m, axis=1)
    return out

